# revision 52
# baseline (speedup 1.0000x reference)
"""Trainium2 Bass kernel for nn_Attention_54391465836966 (v3).

Math (per batch b):
  ctok = content_feat[b].reshape(S,C) + pos            # [1024, 512]
  comp_tok[n] = components[n,b].reshape(S,C) + pos
  q = ctok @ Wq ; k[n],v[n] = comp_tok[n] @ Wkv (split)
  per head h, comp n: P = exp(scale * q k^T); o_nh = (P @ v_nh) / rowsum(P)
  result = sum_n o_n ; s = (result + ctok) @ Wproj + bproj
  out = Wconv[:,:C] @ s2d + Wconv[:,C:] @ cf2d + bconv

s2d = s.reshape(C,S) is a RAW reinterpret: s2d[c2, 512u+w] = s[2*c2+u, w].
So the conv contracts TOKEN PAIRS, giving the exact factorization
  out1[o, 512u+w] = sum_j (W1 @ X[u::2])[o, j] * Wproj[j, w],  X = result+ctok
which shards over j (each core's 128 head-pair channels) with no transposes:
the attention output is accumulated in q-blocks of stride-2 tokens
(block beta = 4u+kappa holds tokens q = 256*kappa + 2p + u), so each result
block is directly the lhsT [c2-chunk, j] of the token-contraction matmul.

Sharding: 8 cores <- (b, hg); per core: q/k/v for 2 heads x 4 components,
attention (q-major, rowsum via ones-column matmul), per-component norm on
DVE, tail T1 (token-contraction, + ctok part) and T2 (x Wproj slice), plus a
quarter of the cf2d conv (second output, placed by host).  Host sums 4
partials per batch and adds weight-only constants (pos/bias folds).

All matmuls bf16 (1 cycle/row in the cost model at any free size; f32 PSUM).
"""
import sys

sys.path.insert(0, "/opt/trn_rl_repo")

import numpy as np
import ml_dtypes

N_CORES = 8
B, C, H, W = 2, 512, 32, 32
S = H * W  # 1024
NH, HD = 8, 64
NC = 4
SCALE = HD ** -0.5

_CACHE = {}


def _build():
    if "nc" in _CACHE:
        return _CACHE["nc"]
    from contextlib import ExitStack

    import concourse.bacc as bacc
    import concourse.mybir as mybir
    import concourse.tile as tile

    f32 = mybir.dt.float32
    f32r = mybir.dt.float32r
    bf16 = mybir.dt.bfloat16
    i16 = mybir.dt.int16
    EXP = mybir.ActivationFunctionType.Exp
    MULT = mybir.AluOpType.mult
    ADD = mybir.AluOpType.add
    # bf16 Schraudolph exp: int16 bits of (t*2^23/ln2 + B)/2^16, bitcast bf16
    EXPA = (2 ** 23 / np.log(2)) * SCALE / 65536.0
    EXPB = (127 * 2 ** 23 - 366393) / 65536.0

    nc = bacc.Bacc("TRN2", target_bir_lowering=False, debug=False,
                   num_devices=N_CORES)

    din = lambda n, s, dt: nc.dram_tensor(n, s, dt, kind="ExternalInput").ap()
    # inputs pre-packed host-side to the exact SBUF tile layout [128, X]
    cfph = [din(f"cfp{h}", [128, 4 * 512], bf16) for h in range(2)]
    comp0h = [din(f"comp0{h}", [128, 4 * 512], bf16) for h in range(2)]
    compp = [None] + [din(f"compp{n}", [128, 4 * S], bf16)
                      for n in range(1, NC)]
    cfup = din("cfup", [128, 8 * 128], bf16)      # cf_tok strided blocks
    cf2dqp = din("cf2dqp", [128, 4 * 256], bf16)  # cf2d quarter cols
    wqp = din("wqp", [128, 4 * 128], bf16)
    wkp = din("wkp", [128, 4 * 128], bf16)
    wvp = din("wvp", [128, 4 * 128], bf16)
    posqp = din("posqp", [128, S], bf16)
    poskp = din("poskp", [128, S], bf16)
    posvp = din("posvp", [128, S], bf16)
    w1tp = din("w1tp", [128, 4 * C], bf16)        # Wconv[:, :C].T packed
    w2tqp = din("w2tqp", [128, 4 * C], bf16)      # Wconv[:, C:].T packed
    wpslicep = din("wpslicep", [128, C], bf16)    # Wproj[sl, :]
    out_p = nc.dram_tensor("out_p", [C, S], bf16, kind="ExternalOutput").ap()
    qout = nc.dram_tensor("qout", [128, 4 * 256], bf16,
                          kind="ExternalOutput").ap()

    with tile.TileContext(nc) as tc, ExitStack() as ctx:
        main = ctx.enter_context(tc.tile_pool(name="main", bufs=1))
        rot = ctx.enter_context(tc.tile_pool(name="rot", bufs=2))
        ps1 = ctx.enter_context(tc.tile_pool(name="ps1", bufs=1, space="PSUM"))
        ps2 = ctx.enter_context(tc.tile_pool(name="ps2", bufs=2, space="PSUM"))

        # ---- constants ----
        ones32 = main.tile([128, 1], f32, tag="o32")
        nc.gpsimd.memset(ones32[:], 1.0)
        ones_b = main.tile([128, 1], bf16, tag="ob")
        nc.vector.tensor_copy(ones_b[:], ones32[:])
        # warm the ACT exp table before the first scores arrive
        warm = main.tile([1, 1], f32, tag="warm")
        nc.scalar.activation(warm[:], ones32[0:1, 0:1], EXP, scale=1.0)

        # ---- input tiles + DMAs (emission order = transfer priority) ----
        wk_sb = main.tile([128, 4 * 128], bf16, tag="wk")
        wq_sb = main.tile([128, 4 * 128], bf16, tag="wq")
        posk_sb = main.tile([128, S], bf16, tag="posk")
        posq_sb = main.tile([128, S], bf16, tag="posq")
        comp_sb = [None] + [main.tile([128, 4 * S], bf16, tag=f"comp{n}",
                                      name=f"comp{n}") for n in range(1, NC)]
        c0h_sb = [main.tile([128, 4 * 512], bf16, tag=f"c0h{h}",
                            name=f"c0h{h}") for h in range(2)]
        cfh_sb = [main.tile([128, 4 * 512], bf16, tag=f"cfh{h}",
                            name=f"cfh{h}") for h in range(2)]
        wv_sb = main.tile([128, 4 * 128], bf16, tag="wv")
        posv_sb = main.tile([128, S], bf16, tag="posv")
        w1t_sb = main.tile([128, 4 * C], bf16, tag="w1t")
        w2tq_sb = main.tile([128, 4 * C], bf16, tag="w2tq")
        wp_sb = main.tile([128, C], bf16, tag="wp")
        cfu_sb = main.tile([128, 8 * 128], bf16, tag="cfu")
        cf2dq_sb = main.tile([128, 4 * 256], bf16, tag="cf2dq")

        nc.sync.dma_start(wk_sb[:], wkp[:])
        nc.sync.dma_start(wq_sb[:], wqp[:])
        nc.sync.dma_start(c0h_sb[0][:], comp0h[0][:])
        nc.sync.dma_start(c0h_sb[1][:], comp0h[1][:])
        nc.sync.dma_start(posk_sb[:], poskp[:])
        nc.sync.dma_start(posq_sb[:], posqp[:])
        nc.sync.dma_start(cfh_sb[0][:], cfph[0][:])
        nc.sync.dma_start(cfh_sb[1][:], cfph[1][:])
        nc.sync.dma_start(wv_sb[:], wvp[:])
        nc.sync.dma_start(posv_sb[:], posvp[:])
        for n in range(1, NC):
            nc.sync.dma_start(comp_sb[n][:], compp[n][:])
        nc.sync.dma_start(w2tq_sb[:], w2tqp[:])
        nc.sync.dma_start(cf2dq_sb[:], cf2dqp[:])
        nc.sync.dma_start(w1t_sb[:], w1tp[:])
        nc.sync.dma_start(cfu_sb[:], cfup[:])
        nc.sync.dma_start(wp_sb[:], wpslicep[:])

        # ---- persistent attention tensors ----
        qT_sb = main.tile([128, S], bf16, tag="qT")
        kT_sb = [main.tile([128, S], bf16, tag=f"kT{n}", name=f"kT{n}")
                 for n in range(NC)]
        v_sb = [main.tile([128, S], bf16, tag=f"v{n}", name=f"v{n}")
                for n in range(NC)]
        res_all = main.tile([128, S], bf16, tag="resall")
        yt_sb = main.tile([128, S], bf16, tag="yt")
        qsum_sb = main.tile([128, 4 * 256], bf16, tag="qsum")

        # PE p-state warmup: ~4us of dummy matmuls so the real stream starts
        # at full clock (ramp needs >3us of continuous execution)
        scratch = main.tile([128, 512], bf16, tag="scratch")
        nc.gpsimd.memset(scratch[:], 0.0)
        dacc = ps1.tile([128, S], f32, tag="acc")
        for _ in range(9):
            nc.tensor.matmul(dacc[:, 0:512], scratch[:, 0:128], scratch[:],
                             start=True, stop=True)

        # kT0 then qT (they gate the first scores); sh-split inputs let each
        # half's matmuls and evac pipeline against the incoming DMA
        acc = ps1.tile([128, S], f32, tag="acc")
        for sh in range(2):
            for k in range(4):
                nc.tensor.matmul(
                    acc[:, 512 * sh:512 * (sh + 1)],
                    wk_sb[:, 128 * k:128 * (k + 1)],
                    c0h_sb[sh][:, 512 * k:512 * (k + 1)],
                    start=(k == 0), stop=(k == 3))
            nc.vector.tensor_add(kT_sb[0][:, 512 * sh:512 * (sh + 1)],
                                 acc[:, 512 * sh:512 * (sh + 1)],
                                 posk_sb[:, 512 * sh:512 * (sh + 1)])
        acc = ps1.tile([128, S], f32, tag="acc")
        for sh in range(2):
            for k in range(4):
                nc.tensor.matmul(
                    acc[:, 512 * sh:512 * (sh + 1)],
                    wq_sb[:, 128 * k:128 * (k + 1)],
                    cfh_sb[sh][:, 512 * k:512 * (k + 1)],
                    start=(k == 0), stop=(k == 3))
            nc.vector.tensor_add(qT_sb[:, 512 * sh:512 * (sh + 1)],
                                 acc[:, 512 * sh:512 * (sh + 1)],
                                 posq_sb[:, 512 * sh:512 * (sh + 1)])

        # ---- filler groups: one per slot, a few matmuls per kt-iteration ----
        def comp_ap(n, k, t):
            # [128, 128] slice: component n, c-chunk k, s-chunk t
            if n == 0:
                h, tt = t // 4, t % 4
                return c0h_sb[h][:, 512 * k + 128 * tt:512 * k + 128 * (tt + 1)]
            return comp_sb[n][:, S * k + 128 * t:S * k + 128 * (t + 1)]

        def group_v(n):
            accv = ps1.tile([128, S], f32, tag="acc")
            ops = []
            for t in range(8):
                for k in range(4):
                    ops.append((lambda t=t, k=k: nc.tensor.matmul(
                        accv[:, 128 * t:128 * (t + 1)],
                        comp_ap(n, k, t),
                        wv_sb[:, 128 * k:128 * (k + 1)],
                        start=(k == 0), stop=(k == 3))))
            ops.append(lambda: nc.vector.tensor_add(v_sb[n][:], accv[:],
                                                    posv_sb[:]))
            return ops

        def group_kT(n):
            acck = ps1.tile([128, S], f32, tag="acc")
            ops = []
            for sh in range(2):
                for k in range(4):
                    ops.append((lambda sh=sh, k=k: nc.tensor.matmul(
                        acck[:, 512 * sh:512 * (sh + 1)],
                        wk_sb[:, 128 * k:128 * (k + 1)],
                        comp_sb[n][:, S * k + 512 * sh:S * k + 512 * (sh + 1)],
                        start=(k == 0), stop=(k == 3))))
            ops.append(lambda: nc.vector.tensor_add(kT_sb[n][:], acck[:],
                                                    posk_sb[:]))
            return ops

        def group_quarters():
            accq = ps1.tile([128, S], f32, tag="acc")
            ops = []
            for m in range(4):
                for k in range(4):
                    ops.append((lambda m=m, k=k: nc.tensor.matmul(
                        accq[:, 256 * m:256 * (m + 1)],
                        w2tq_sb[:, 512 * k + 128 * m:512 * k + 128 * (m + 1)],
                        cf2dq_sb[:, 256 * k:256 * (k + 1)],
                        start=(k == 0), stop=(k == 3))))

            def fin():
                nc.vector.tensor_copy(qsum_sb[:], accq[:])
                nc.sync.dma_start(qout[:], qsum_sb[:])
            ops.append(fin)
            return ops

        slot_groups = {
            0: ("v", 0), 1: ("kT", 1), 2: ("v", 1), 3: ("kT", 2),
            4: ("v", 2), 5: ("kT", 3), 6: ("v", 3), 7: ("q", 0),
        }

        def pblk(P_t, kt, beta):
            # strided token block: cols S*kt + 256*(beta%4) + 2*qq + beta//4
            kap, u = beta % 4, beta // 4
            base = S * kt + 256 * kap
            return P_t[:, base:base + 256].rearrange(
                "p (q two) -> p two q", two=2)[:, u, :]

        def res_dst(hp_prev, b0, nb):
            # res_all cols 128*beta + 64*hp + e for beta in [b0, b0+nb)
            return res_all[:].rearrange("p (b two e) -> p b two e",
                                        b=8, two=2)[:, b0:b0 + nb, hp_prev, :]

        def norm_half(po, n_prev, hp_prev, b0, zr):
            # batched normalization for beta in [b0, b0+4): one recip, one
            # broadcast multiply, one add (n_prev > 0)
            nc.vector.reciprocal(zr[:, b0:b0 + 4], po[:, 512 + b0:516 + b0])
            zbc = zr[:, b0:b0 + 4].unsqueeze(2).to_broadcast([128, 4, 64])
            src_ap = po[:, 64 * b0:64 * (b0 + 4)].rearrange(
                "p (b e) -> p b e", b=4)
            if n_prev == 0:
                nc.vector.tensor_mul(res_dst(hp_prev, b0, 4), src_ap, zbc)
            else:
                otmp = _CACHE["otmp"]
                ot = otmp[:, 64 * b0:64 * (b0 + 4)].rearrange(
                    "p (b e) -> p b e", b=4)
                nc.vector.tensor_mul(ot, src_ap, zbc)
                nc.gpsimd.tensor_add(res_dst(hp_prev, b0, 4),
                                     res_dst(hp_prev, b0, 4), ot)

        # ---- attention pipeline: 8 slots; slot s runs o-matmuls of s-1 ----
        prev = None  # (P_tile, n, hp)
        for s in range(8):
            n, hp = s // 2, s % 2
            P_cur = rot.tile([128, 8 * S], bf16, tag="P")
            if prev is not None:
                po = ps1.tile([128, S], f32, tag="o")
                zr = rot.tile([128, 8], f32, tag="zr")
                otmp_t = rot.tile([128, 512], bf16, tag="otmp", name="otmp")
                _CACHE["otmp"] = otmp_t
            gname = slot_groups.get(s)
            if gname is not None:
                kind, gn = gname
                ops = (group_v(gn) if kind == "v" else
                       group_kT(gn) if kind == "kT" else group_quarters())
            else:
                ops = []
            per_iter = (len(ops) + 7) // 8
            for i in range(8):
                sc = ps2.tile([128, S], f32, tag="sc")
                for qc2 in range(2):
                    nc.tensor.matmul(
                        sc[:, 512 * qc2:512 * (qc2 + 1)],
                        kT_sb[n][64 * hp:64 * hp + 64,
                                 128 * i:128 * (i + 1)],
                        qT_sb[64 * hp:64 * hp + 64,
                              512 * qc2:512 * (qc2 + 1)],
                        start=True, stop=True)
                if i in (0, 3, 6):
                    # offload 2 of 8 exps per slot to DVE via the bf16
                    # Schraudolph bit-trick (ACT is the critical engine)
                    nc.vector.tensor_scalar(
                        P_cur[:, S * i:S * (i + 1)].bitcast(i16), sc[:],
                        EXPA, EXPB, MULT, ADD)
                else:
                    nc.scalar.activation(P_cur[:, S * i:S * (i + 1)], sc[:],
                                         EXP, scale=SCALE)
                if prev is not None:
                    Pp, pn, php = prev
                    for kt in range(8):
                        nc.tensor.matmul(
                            po[:, 64 * i:64 * (i + 1)],
                            pblk(Pp, kt, i),
                            v_sb[pn][:, 128 * kt + 64 * php:
                                     128 * kt + 64 * php + 64],
                            start=(kt == 0), stop=(kt == 7))
                        nc.tensor.matmul(
                            po[:, 512 + i:513 + i],
                            pblk(Pp, kt, i),
                            ones_b[:],
                            start=(kt == 0), stop=(kt == 7))
                for _ in range(per_iter):
                    if ops:
                        ops.pop(0)()
                if prev is not None and i == 6:
                    norm_half(po, pn, php, 0, zr)
            while ops:
                ops.pop(0)()
            if prev is not None:
                norm_half(po, prev[1], prev[2], 4, zr)
            prev = (P_cur, n, hp)

        # ---- drain: slot 7 o-matmuls first (critical chain), then ctok
        # T1 as filler, with the normalization split into halves so each
        # yt half evacuates as soon as its blocks are ready ----
        Pp, pn, php = prev
        po = ps1.tile([128, S], f32, tag="o")
        zr = rot.tile([128, 8], f32, tag="zr")
        otmp_d = rot.tile([128, 512], bf16, tag="otmp", name="otmpd")
        yt_acc = ps1.tile([128, S], f32, tag="acc")

        def drain_norm_half(b0):
            nc.vector.reciprocal(zr[:, b0:b0 + 4], po[:, 512 + b0:516 + b0])
            zbc = zr[:, b0:b0 + 4].unsqueeze(2).to_broadcast([128, 4, 64])
            srch = po[:, 64 * b0:64 * (b0 + 4)].rearrange(
                "p (b e) -> p b e", b=4)
            oth = otmp_d[:, 64 * b0:64 * (b0 + 4)].rearrange(
                "p (b e) -> p b e", b=4)
            nc.vector.tensor_mul(oth, srch, zbc)
            nc.vector.tensor_add(res_dst(php, b0, 4), res_dst(php, b0, 4),
                                 oth)

        # beta-outer: one open accumulation group per PSUM bank at a time
        # (interleaving multiple open groups within a bank corrupts the
        # accumulation on hardware)
        def drain_obeta(b0):
            for beta in range(b0, b0 + 4):
                for kt in range(8):
                    nc.tensor.matmul(
                        po[:, 64 * beta:64 * (beta + 1)],
                        pblk(Pp, kt, beta),
                        v_sb[pn][:, 128 * kt + 64 * php:
                                 128 * kt + 64 * php + 64],
                        start=(kt == 0), stop=(kt == 7))
                    nc.tensor.matmul(
                        po[:, 512 + beta:513 + beta],
                        pblk(Pp, kt, beta),
                        ones_b[:],
                        start=(kt == 0), stop=(kt == 7))

        drain_obeta(0)
        drain_obeta(4)
        # ctok part of T1 opens both u-region accumulation groups; emitted
        # after the o-matmuls so it fills PE time while the norm runs
        for u in range(2):
            for kap in range(4):
                nc.tensor.matmul(
                    yt_acc[:, 512 * u:512 * (u + 1)],
                    cfu_sb[:, 128 * (4 * u + kap):128 * (4 * u + kap + 1)],
                    w1t_sb[:, C * kap:C * (kap + 1)],
                    start=(kap == 0), stop=False)
        drain_norm_half(0)
        drain_norm_half(4)
        for half in range(2):
            for kap in range(4):
                beta = 4 * half + kap
                nc.tensor.matmul(yt_acc[:, 512 * half:512 * (half + 1)],
                                 res_all[:, 128 * beta:128 * (beta + 1)],
                                 w1t_sb[:, C * kap:C * (kap + 1)],
                                 start=False, stop=(kap == 3))
            if half == 0:
                nc.vector.tensor_copy(yt_sb[:, 0:512], yt_acc[:, 0:512])
            else:
                nc.scalar.copy(yt_sb[:, 512:1024], yt_acc[:, 512:1024])

        # ---- tail T2: out1[o-chunk, 512u+w] = Yt_u[:, o-chunk]^T @ Wproj[sl]
        for m in range(4):
            if m < 2:
                accf = ps2.tile([128, S], f32, tag="sc", name=f"accf{m}")
            elif m == 2:
                accf = ps1.tile([128, S], f32, tag="o", name="accf2")
            else:
                accf = ps1.tile([128, S], f32, tag="acc", name="accf3")
            for u in range(2):
                nc.tensor.matmul(
                    accf[:, 512 * u:512 * (u + 1)],
                    yt_sb[:, 512 * u + 128 * m:512 * u + 128 * (m + 1)],
                    wp_sb[:],
                    start=True, stop=True)
            out_m = rot.tile([128, S], bf16, tag="outsb", bufs=4,
                             name=f"out{m}")
            if m % 2 == 0:
                nc.scalar.copy(out_m[:], accf[:])
            else:
                nc.vector.tensor_copy(out_m[:], accf[:])
            nc.sync.dma_start(out_p[128 * m:128 * (m + 1), :], out_m[:])

    nc.compile()
    _CACHE.pop("otmp", None)
    _CACHE["nc"] = nc
    return nc


def _pack4(a, w):
    # [4*128, w] -> [128, 4*w] with chunk k at cols [k*w, (k+1)*w)
    return np.ascontiguousarray(
        a.reshape(4, 128, w).transpose(1, 0, 2).reshape(128, 4 * w))


def _shard_inputs(content_feat, components, pos_emb, Wq, Wkv, Wproj, bproj,
                  Wconv, bconv):
    f = np.float32
    bf = ml_dtypes.bfloat16
    pos2 = np.asarray(pos_emb, f).reshape(S, C)
    Wq = np.asarray(Wq, f)
    Wkv = np.asarray(Wkv, f)
    Wproj = np.asarray(Wproj, f)
    Wconv = np.asarray(Wconv, f)
    w1t = np.ascontiguousarray(Wconv[:, :C].T)   # [c2, o]
    w2t = np.ascontiguousarray(Wconv[:, C:].T)   # [c_in, o]
    w1tp = _pack4(w1t, C).astype(bf)
    w2tqp = _pack4(w2t, C).astype(bf)

    cfTs, comps, cf2ds, cfts = [], [], [], []
    for b in range(B):
        ct = np.asarray(content_feat[b], f).reshape(S, C)
        cfts.append(ct)
        cfTs.append(np.ascontiguousarray(ct.T))
        cf2ds.append(np.asarray(content_feat[b], f).reshape(C, S))
        comps.append([np.ascontiguousarray(
            np.asarray(components[n, b], f).reshape(S, C).T)
            for n in range(NC)])

    in_maps = []
    for core in range(N_CORES):
        b, hg = core // 4, core % 4
        sl = slice(128 * hg, 128 * (hg + 1))
        vsl = slice(C + 128 * hg, C + 128 * (hg + 1))
        qsl = slice(256 * hg, 256 * (hg + 1))
        posv = pos2 @ Wkv[:, vsl]            # [S, 128]
        # cf_tok strided blocks for the ctok tail: block 4u+kap is
        # cf_tok[u::2][128kap:128kap+128, sl]
        cfu = np.zeros((128, 8 * 128), f)
        for u in range(2):
            half = cfts[b][u::2]             # [512, C]
            for kap in range(4):
                cfu[:, 128 * (4 * u + kap):128 * (4 * u + kap + 1)] = \
                    half[128 * kap:128 * (kap + 1), sl]
        in_maps.append({
            **{f"cfp{h}": _pack4(np.ascontiguousarray(
                cfTs[b][:, 512 * h:512 * (h + 1)]), 512).astype(bf)
               for h in range(2)},
            **{f"comp0{h}": _pack4(np.ascontiguousarray(
                comps[b][0][:, 512 * h:512 * (h + 1)]), 512).astype(bf)
               for h in range(2)},
            **{f"compp{n}": _pack4(comps[b][n], S).astype(bf)
               for n in range(1, NC)},
            "cfup": cfu.astype(bf),
            "cf2dqp": _pack4(np.ascontiguousarray(cf2ds[b][:, qsl]),
                             256).astype(bf),
            "wqp": _pack4(np.ascontiguousarray(Wq[:, sl]), 128).astype(bf),
            "wkp": _pack4(np.ascontiguousarray(Wkv[:, sl]), 128).astype(bf),
            "wvp": _pack4(np.ascontiguousarray(Wkv[:, vsl]), 128).astype(bf),
            "posqp": np.ascontiguousarray((pos2 @ Wq[:, sl]).T).astype(bf),
            "poskp": np.ascontiguousarray((pos2 @ Wkv[:, sl]).T).astype(bf),
            "posvp": np.ascontiguousarray(
                posv.reshape(8, 128, 128).transpose(1, 0, 2)
                .reshape(128, S)).astype(bf),
            "w1tp": w1tp,
            "w2tqp": w2tqp,
            "wpslicep": np.ascontiguousarray(Wproj[sl, :]).astype(bf),
        })
    return in_maps


def _gather(res, inputs):
    f = np.float32
    Wconv = np.asarray(inputs["Wconv"], f)
    Wproj = np.asarray(inputs["Wproj"], f)
    bproj = np.asarray(inputs["bproj"], f)
    bconv = np.asarray(inputs["bconv"], f)
    pos2 = np.asarray(inputs["pos_emb"], f).reshape(S, C)
    W1 = Wconv[:, :C]
    w1sum = W1.sum(axis=1)
    const = np.zeros((C, S), f)
    for u in range(2):
        const[:, 512 * u:512 * (u + 1)] = \
            (W1 @ pos2[u::2]) @ Wproj + np.outer(w1sum, bproj)
    const += bconv[:, None]
    out = []
    for b in range(B):
        acc = const.copy()
        for hg in range(4):
            r = res.results[4 * b + hg]
            acc = acc + np.asarray(r["out_p"], f)
            q = np.asarray(r["qout"], f).reshape(128, 4, 256)
            acc[:, 256 * hg:256 * (hg + 1)] += (
                q.transpose(1, 0, 2).reshape(512, 256))
        out.append(acc)
    return np.stack(out, axis=0).reshape(B, C, H, W).astype(np.float32)


def _run(trace=False, **inputs):
    from concourse.bass_utils import run_bass_kernel_spmd

    nc = _build()
    in_maps = _shard_inputs(**inputs)
    res = run_bass_kernel_spmd(nc, in_maps, list(range(N_CORES)), trace=trace)
    return _gather(res, inputs), res


def kernel(**inputs):
    out, _ = _run(trace=False, **inputs)
    return out


# revision 53
# speedup vs baseline: 1.0336x; 1.0336x over previous
"""Trainium2 Bass kernel for nn_Attention_54391465836966 (v3).

Math (per batch b):
  ctok = content_feat[b].reshape(S,C) + pos            # [1024, 512]
  comp_tok[n] = components[n,b].reshape(S,C) + pos
  q = ctok @ Wq ; k[n],v[n] = comp_tok[n] @ Wkv (split)
  per head h, comp n: P = exp(scale * q k^T); o_nh = (P @ v_nh) / rowsum(P)
  result = sum_n o_n ; s = (result + ctok) @ Wproj + bproj
  out = Wconv[:,:C] @ s2d + Wconv[:,C:] @ cf2d + bconv

s2d = s.reshape(C,S) is a RAW reinterpret: s2d[c2, 512u+w] = s[2*c2+u, w].
So the conv contracts TOKEN PAIRS, giving the exact factorization
  out1[o, 512u+w] = sum_j (W1 @ X[u::2])[o, j] * Wproj[j, w],  X = result+ctok
which shards over j (each core's 128 head-pair channels) with no transposes:
the attention output is accumulated in q-blocks of stride-2 tokens
(block beta = 4u+kappa holds tokens q = 256*kappa + 2p + u), so each result
block is directly the lhsT [c2-chunk, j] of the token-contraction matmul.

Sharding: 8 cores <- (b, hg); per core: q/k/v for 2 heads x 4 components,
attention (q-major, rowsum via ones-column matmul), per-component norm on
DVE, tail T1 (token-contraction, + ctok part) and T2 (x Wproj slice), plus a
quarter of the cf2d conv (second output, placed by host).  Host sums 4
partials per batch and adds weight-only constants (pos/bias folds).

All matmuls bf16 (1 cycle/row in the cost model at any free size; f32 PSUM).
"""
import sys

sys.path.insert(0, "/opt/trn_rl_repo")

import numpy as np
import ml_dtypes

N_CORES = 8
B, C, H, W = 2, 512, 32, 32
S = H * W  # 1024
NH, HD = 8, 64
NC = 4
SCALE = HD ** -0.5

_CACHE = {}


def _build():
    if "nc" in _CACHE:
        return _CACHE["nc"]
    from contextlib import ExitStack

    import concourse.bacc as bacc
    import concourse.mybir as mybir
    import concourse.tile as tile

    f32 = mybir.dt.float32
    f32r = mybir.dt.float32r
    bf16 = mybir.dt.bfloat16
    i16 = mybir.dt.int16
    EXP = mybir.ActivationFunctionType.Exp
    MULT = mybir.AluOpType.mult
    ADD = mybir.AluOpType.add
    # bf16 Schraudolph exp: int16 bits of (t*2^23/ln2 + B)/2^16, bitcast bf16
    EXPA = (2 ** 23 / np.log(2)) * SCALE / 65536.0
    EXPB = (127 * 2 ** 23 - 366393) / 65536.0

    nc = bacc.Bacc("TRN2", target_bir_lowering=False, debug=False,
                   num_devices=N_CORES)

    din = lambda n, s, dt: nc.dram_tensor(n, s, dt, kind="ExternalInput").ap()
    # inputs pre-packed host-side to the exact SBUF tile layout [128, X]
    cfph = [din(f"cfp{h}", [128, 4 * 512], bf16) for h in range(2)]
    comp0h = [din(f"comp0{h}", [128, 4 * 512], bf16) for h in range(2)]
    compp = [None] + [din(f"compp{n}", [128, 4 * S], bf16)
                      for n in range(1, NC)]
    cfup = din("cfup", [128, 8 * 128], bf16)      # cf_tok strided blocks
    cf2dqp = din("cf2dqp", [128, 4 * 256], bf16)  # cf2d quarter cols
    wqp = din("wqp", [128, 4 * 128], bf16)
    wkp = din("wkp", [128, 4 * 128], bf16)
    wvp = din("wvp", [128, 4 * 128], bf16)
    posqp = din("posqp", [128, S], bf16)
    poskp = din("poskp", [128, S], bf16)
    posvp = din("posvp", [128, S], bf16)
    w1tp = din("w1tp", [128, 4 * C], bf16)        # Wconv[:, :C].T packed
    w2tqp = din("w2tqp", [128, 4 * C], bf16)      # Wconv[:, C:].T packed
    wpslicep = din("wpslicep", [128, C], bf16)    # Wproj[sl, :]
    out_p = nc.dram_tensor("out_p", [C, S], bf16, kind="ExternalOutput").ap()
    qout = nc.dram_tensor("qout", [128, 4 * 256], bf16,
                          kind="ExternalOutput").ap()

    with tile.TileContext(nc) as tc, ExitStack() as ctx:
        main = ctx.enter_context(tc.tile_pool(name="main", bufs=1))
        rot = ctx.enter_context(tc.tile_pool(name="rot", bufs=2))
        ps1 = ctx.enter_context(tc.tile_pool(name="ps1", bufs=1, space="PSUM"))
        ps2 = ctx.enter_context(tc.tile_pool(name="ps2", bufs=2, space="PSUM"))

        # ---- constants ----
        ones32 = main.tile([128, 1], f32, tag="o32")
        nc.gpsimd.memset(ones32[:], 1.0)
        ones_b = main.tile([128, 1], bf16, tag="ob")
        nc.vector.tensor_copy(ones_b[:], ones32[:])
        # warm the ACT exp table before the first scores arrive
        warm = main.tile([1, 1], f32, tag="warm")
        nc.scalar.activation(warm[:], ones32[0:1, 0:1], EXP, scale=1.0)

        # ---- input tiles + DMAs (emission order = transfer priority) ----
        wk_sb = main.tile([128, 4 * 128], bf16, tag="wk")
        wq_sb = main.tile([128, 4 * 128], bf16, tag="wq")
        posk_sb = main.tile([128, S], bf16, tag="posk")
        posq_sb = main.tile([128, S], bf16, tag="posq")
        comp_sb = [None] + [main.tile([128, 4 * S], bf16, tag=f"comp{n}",
                                      name=f"comp{n}") for n in range(1, NC)]
        c0h_sb = [main.tile([128, 4 * 512], bf16, tag=f"c0h{h}",
                            name=f"c0h{h}") for h in range(2)]
        cfh_sb = [main.tile([128, 4 * 512], bf16, tag=f"cfh{h}",
                            name=f"cfh{h}") for h in range(2)]
        wv_sb = main.tile([128, 4 * 128], bf16, tag="wv")
        posv_sb = main.tile([128, S], bf16, tag="posv")
        w1t_sb = main.tile([128, 4 * C], bf16, tag="w1t")
        w2tq_sb = main.tile([128, 4 * C], bf16, tag="w2tq")
        wp_sb = main.tile([128, C], bf16, tag="wp")
        cfu_sb = main.tile([128, 8 * 128], bf16, tag="cfu")
        cf2dq_sb = main.tile([128, 4 * 256], bf16, tag="cf2dq")

        nc.sync.dma_start(wk_sb[:], wkp[:])
        nc.sync.dma_start(wq_sb[:], wqp[:])
        nc.sync.dma_start(c0h_sb[0][:], comp0h[0][:])
        nc.sync.dma_start(c0h_sb[1][:], comp0h[1][:])
        nc.sync.dma_start(posk_sb[:], poskp[:])
        nc.sync.dma_start(posq_sb[:], posqp[:])
        nc.sync.dma_start(cfh_sb[0][:], cfph[0][:])
        nc.sync.dma_start(cfh_sb[1][:], cfph[1][:])
        nc.sync.dma_start(wv_sb[:], wvp[:])
        nc.sync.dma_start(posv_sb[:], posvp[:])
        for n in range(1, NC):
            nc.sync.dma_start(comp_sb[n][:], compp[n][:])
        nc.sync.dma_start(w2tq_sb[:], w2tqp[:])
        nc.sync.dma_start(cf2dq_sb[:], cf2dqp[:])
        nc.sync.dma_start(w1t_sb[:], w1tp[:])
        nc.sync.dma_start(cfu_sb[:], cfup[:])
        nc.sync.dma_start(wp_sb[:], wpslicep[:])

        # ---- persistent attention tensors ----
        qT_sb = main.tile([128, S], bf16, tag="qT")
        kT_sb = [main.tile([128, S], bf16, tag=f"kT{n}", name=f"kT{n}")
                 for n in range(NC)]
        v_sb = [main.tile([128, S], bf16, tag=f"v{n}", name=f"v{n}")
                for n in range(NC)]
        res_all = main.tile([128, S], bf16, tag="resall")
        yt_sb = main.tile([128, S], bf16, tag="yt")
        qsum_sb = main.tile([128, 4 * 256], bf16, tag="qsum")

        # PE p-state warmup: ~4us of dummy matmuls so the real stream starts
        # at full clock (ramp needs >3us of continuous execution)
        scratch = main.tile([128, 512], bf16, tag="scratch")
        nc.gpsimd.memset(scratch[:], 0.0)
        dacc = ps1.tile([128, S], f32, tag="acc")
        for _ in range(9):
            nc.tensor.matmul(dacc[:, 0:512], scratch[:, 0:128], scratch[:],
                             start=True, stop=True)

        # kT0 then qT (they gate the first scores); sh-split inputs let each
        # half's matmuls and evac pipeline against the incoming DMA
        acc = ps1.tile([128, S], f32, tag="acc")
        for sh in range(2):
            for k in range(4):
                nc.tensor.matmul(
                    acc[:, 512 * sh:512 * (sh + 1)],
                    wk_sb[:, 128 * k:128 * (k + 1)],
                    c0h_sb[sh][:, 512 * k:512 * (k + 1)],
                    start=(k == 0), stop=(k == 3))
            nc.vector.tensor_add(kT_sb[0][:, 512 * sh:512 * (sh + 1)],
                                 acc[:, 512 * sh:512 * (sh + 1)],
                                 posk_sb[:, 512 * sh:512 * (sh + 1)])
        acc = ps1.tile([128, S], f32, tag="acc")
        for sh in range(2):
            for k in range(4):
                nc.tensor.matmul(
                    acc[:, 512 * sh:512 * (sh + 1)],
                    wq_sb[:, 128 * k:128 * (k + 1)],
                    cfh_sb[sh][:, 512 * k:512 * (k + 1)],
                    start=(k == 0), stop=(k == 3))
            nc.vector.tensor_add(qT_sb[:, 512 * sh:512 * (sh + 1)],
                                 acc[:, 512 * sh:512 * (sh + 1)],
                                 posq_sb[:, 512 * sh:512 * (sh + 1)])

        # ---- filler groups: one per slot, a few matmuls per kt-iteration ----
        def comp_ap(n, k, t):
            # [128, 128] slice: component n, c-chunk k, s-chunk t
            if n == 0:
                h, tt = t // 4, t % 4
                return c0h_sb[h][:, 512 * k + 128 * tt:512 * k + 128 * (tt + 1)]
            return comp_sb[n][:, S * k + 128 * t:S * k + 128 * (t + 1)]

        def group_v(n):
            accv = ps1.tile([128, S], f32, tag="acc")
            ops = []
            for t in range(8):
                for k in range(4):
                    ops.append((lambda t=t, k=k: nc.tensor.matmul(
                        accv[:, 128 * t:128 * (t + 1)],
                        comp_ap(n, k, t),
                        wv_sb[:, 128 * k:128 * (k + 1)],
                        start=(k == 0), stop=(k == 3))))
            ops.append(lambda: nc.vector.tensor_add(v_sb[n][:], accv[:],
                                                    posv_sb[:]))
            return ops

        def group_kT(n):
            acck = ps1.tile([128, S], f32, tag="acc")
            ops = []
            for sh in range(2):
                for k in range(4):
                    ops.append((lambda sh=sh, k=k: nc.tensor.matmul(
                        acck[:, 512 * sh:512 * (sh + 1)],
                        wk_sb[:, 128 * k:128 * (k + 1)],
                        comp_sb[n][:, S * k + 512 * sh:S * k + 512 * (sh + 1)],
                        start=(k == 0), stop=(k == 3))))
            ops.append(lambda: nc.vector.tensor_add(kT_sb[n][:], acck[:],
                                                    posk_sb[:]))
            return ops

        def group_quarters():
            accq = ps1.tile([128, S], f32, tag="acc")
            ops = []
            for m in range(4):
                for k in range(4):
                    ops.append((lambda m=m, k=k: nc.tensor.matmul(
                        accq[:, 256 * m:256 * (m + 1)],
                        w2tq_sb[:, 512 * k + 128 * m:512 * k + 128 * (m + 1)],
                        cf2dq_sb[:, 256 * k:256 * (k + 1)],
                        start=(k == 0), stop=(k == 3))))

            def fin():
                nc.vector.tensor_copy(qsum_sb[:], accq[:])
                nc.sync.dma_start(qout[:], qsum_sb[:])
            ops.append(fin)
            return ops

        slot_groups = {
            0: ("v", 0), 1: ("kT", 1), 2: ("v", 1), 3: ("kT", 2),
            4: ("v", 2), 5: ("kT", 3), 6: ("v", 3), 7: ("q", 0),
        }

        def pblk(P_t, kt, beta):
            # strided token block: cols S*kt + 256*(beta%4) + 2*qq + beta//4
            kap, u = beta % 4, beta // 4
            base = S * kt + 256 * kap
            return P_t[:, base:base + 256].rearrange(
                "p (q two) -> p two q", two=2)[:, u, :]

        def res_dst(hp_prev, b0, nb):
            # res_all cols 128*beta + 64*hp + e for beta in [b0, b0+nb)
            return res_all[:].rearrange("p (b two e) -> p b two e",
                                        b=8, two=2)[:, b0:b0 + nb, hp_prev, :]

        def norm_half(po, n_prev, hp_prev, b0, zr):
            # batched normalization for beta in [b0, b0+4): one recip, one
            # broadcast multiply, one add (n_prev > 0)
            nc.vector.reciprocal(zr[:, b0:b0 + 4], po[:, 512 + b0:516 + b0])
            zbc = zr[:, b0:b0 + 4].unsqueeze(2).to_broadcast([128, 4, 64])
            src_ap = po[:, 64 * b0:64 * (b0 + 4)].rearrange(
                "p (b e) -> p b e", b=4)
            if n_prev == 0:
                nc.vector.tensor_mul(res_dst(hp_prev, b0, 4), src_ap, zbc)
            else:
                otmp = _CACHE["otmp"]
                ot = otmp[:, 64 * b0:64 * (b0 + 4)].rearrange(
                    "p (b e) -> p b e", b=4)
                nc.vector.tensor_mul(ot, src_ap, zbc)
                nc.gpsimd.tensor_add(res_dst(hp_prev, b0, 4),
                                     res_dst(hp_prev, b0, 4), ot)

        # ---- attention pipeline: 8 slots; slot s runs o-matmuls of s-1 ----
        prev = None  # (P_tile, n, hp)
        for s in range(8):
            n, hp = s // 2, s % 2
            P_cur = rot.tile([128, 8 * S], bf16, tag="P")
            if prev is not None:
                po = ps1.tile([128, S], f32, tag="o")
                zr = rot.tile([128, 8], f32, tag="zr")
                otmp_t = rot.tile([128, 512], bf16, tag="otmp", name="otmp")
                _CACHE["otmp"] = otmp_t
            gname = slot_groups.get(s)
            if gname is not None:
                kind, gn = gname
                ops = (group_v(gn) if kind == "v" else
                       group_kT(gn) if kind == "kT" else group_quarters())
            else:
                ops = []
            per_iter = (len(ops) + 7) // 8
            for i in range(8):
                sc = ps2.tile([128, S], f32, tag="sc")
                for qc2 in range(2):
                    nc.tensor.matmul(
                        sc[:, 512 * qc2:512 * (qc2 + 1)],
                        kT_sb[n][64 * hp:64 * hp + 64,
                                 128 * i:128 * (i + 1)],
                        qT_sb[64 * hp:64 * hp + 64,
                              512 * qc2:512 * (qc2 + 1)],
                        start=True, stop=True)
                if i in (2, 4, 6):
                    # offload 2 of 8 exps per slot to DVE via the bf16
                    # Schraudolph bit-trick (ACT is the critical engine)
                    nc.vector.tensor_scalar(
                        P_cur[:, S * i:S * (i + 1)].bitcast(i16), sc[:],
                        EXPA, EXPB, MULT, ADD)
                else:
                    nc.scalar.activation(P_cur[:, S * i:S * (i + 1)], sc[:],
                                         EXP, scale=SCALE)
                if prev is not None:
                    Pp, pn, php = prev
                    for kt in range(8):
                        nc.tensor.matmul(
                            po[:, 64 * i:64 * (i + 1)],
                            pblk(Pp, kt, i),
                            v_sb[pn][:, 128 * kt + 64 * php:
                                     128 * kt + 64 * php + 64],
                            start=(kt == 0), stop=(kt == 7))
                        nc.tensor.matmul(
                            po[:, 512 + i:513 + i],
                            pblk(Pp, kt, i),
                            ones_b[:],
                            start=(kt == 0), stop=(kt == 7))
                for _ in range(per_iter):
                    if ops:
                        ops.pop(0)()
                if prev is not None and i == 6:
                    norm_half(po, pn, php, 0, zr)
            while ops:
                ops.pop(0)()
            if prev is not None:
                norm_half(po, prev[1], prev[2], 4, zr)
            prev = (P_cur, n, hp)

        # ---- drain: slot 7 o-matmuls first (critical chain), then ctok
        # T1 as filler, with the normalization split into halves so each
        # yt half evacuates as soon as its blocks are ready ----
        Pp, pn, php = prev
        po = ps1.tile([128, S], f32, tag="o")
        zr = rot.tile([128, 8], f32, tag="zr")
        otmp_d = rot.tile([128, 512], bf16, tag="otmp", name="otmpd")
        yt_acc = ps1.tile([128, S], f32, tag="acc")

        def drain_norm_half(b0):
            nc.vector.reciprocal(zr[:, b0:b0 + 4], po[:, 512 + b0:516 + b0])
            zbc = zr[:, b0:b0 + 4].unsqueeze(2).to_broadcast([128, 4, 64])
            srch = po[:, 64 * b0:64 * (b0 + 4)].rearrange(
                "p (b e) -> p b e", b=4)
            oth = otmp_d[:, 64 * b0:64 * (b0 + 4)].rearrange(
                "p (b e) -> p b e", b=4)
            nc.vector.tensor_mul(oth, srch, zbc)
            nc.vector.tensor_add(res_dst(php, b0, 4), res_dst(php, b0, 4),
                                 oth)

        # beta-outer: one open accumulation group per PSUM bank at a time
        # (interleaving multiple open groups within a bank corrupts the
        # accumulation on hardware)
        def drain_obeta(b0):
            for beta in range(b0, b0 + 4):
                for kt in range(8):
                    nc.tensor.matmul(
                        po[:, 64 * beta:64 * (beta + 1)],
                        pblk(Pp, kt, beta),
                        v_sb[pn][:, 128 * kt + 64 * php:
                                 128 * kt + 64 * php + 64],
                        start=(kt == 0), stop=(kt == 7))
                    nc.tensor.matmul(
                        po[:, 512 + beta:513 + beta],
                        pblk(Pp, kt, beta),
                        ones_b[:],
                        start=(kt == 0), stop=(kt == 7))

        drain_obeta(0)
        drain_obeta(4)
        # ctok part of T1 opens both u-region accumulation groups; emitted
        # after the o-matmuls so it fills PE time while the norm runs
        for u in range(2):
            for kap in range(4):
                nc.tensor.matmul(
                    yt_acc[:, 512 * u:512 * (u + 1)],
                    cfu_sb[:, 128 * (4 * u + kap):128 * (4 * u + kap + 1)],
                    w1t_sb[:, C * kap:C * (kap + 1)],
                    start=(kap == 0), stop=False)
        drain_norm_half(0)
        drain_norm_half(4)
        for half in range(2):
            for kap in range(4):
                beta = 4 * half + kap
                nc.tensor.matmul(yt_acc[:, 512 * half:512 * (half + 1)],
                                 res_all[:, 128 * beta:128 * (beta + 1)],
                                 w1t_sb[:, C * kap:C * (kap + 1)],
                                 start=False, stop=(kap == 3))
            if half == 0:
                nc.vector.tensor_copy(yt_sb[:, 0:512], yt_acc[:, 0:512])
            else:
                nc.scalar.copy(yt_sb[:, 512:1024], yt_acc[:, 512:1024])

        # ---- tail T2: out1[o-chunk, 512u+w] = Yt_u[:, o-chunk]^T @ Wproj[sl]
        for m in range(4):
            if m < 2:
                accf = ps2.tile([128, S], f32, tag="sc", name=f"accf{m}")
            elif m == 2:
                accf = ps1.tile([128, S], f32, tag="o", name="accf2")
            else:
                accf = ps1.tile([128, S], f32, tag="acc", name="accf3")
            for u in range(2):
                nc.tensor.matmul(
                    accf[:, 512 * u:512 * (u + 1)],
                    yt_sb[:, 512 * u + 128 * m:512 * u + 128 * (m + 1)],
                    wp_sb[:],
                    start=True, stop=True)
            out_m = rot.tile([128, S], bf16, tag="outsb", bufs=4,
                             name=f"out{m}")
            if m % 2 == 0:
                nc.scalar.copy(out_m[:], accf[:])
            else:
                nc.vector.tensor_copy(out_m[:], accf[:])
            nc.sync.dma_start(out_p[128 * m:128 * (m + 1), :], out_m[:])

    nc.compile()
    _CACHE.pop("otmp", None)
    _CACHE["nc"] = nc
    return nc


def _pack4(a, w):
    # [4*128, w] -> [128, 4*w] with chunk k at cols [k*w, (k+1)*w)
    return np.ascontiguousarray(
        a.reshape(4, 128, w).transpose(1, 0, 2).reshape(128, 4 * w))


def _shard_inputs(content_feat, components, pos_emb, Wq, Wkv, Wproj, bproj,
                  Wconv, bconv):
    f = np.float32
    bf = ml_dtypes.bfloat16
    pos2 = np.asarray(pos_emb, f).reshape(S, C)
    Wq = np.asarray(Wq, f)
    Wkv = np.asarray(Wkv, f)
    Wproj = np.asarray(Wproj, f)
    Wconv = np.asarray(Wconv, f)
    w1t = np.ascontiguousarray(Wconv[:, :C].T)   # [c2, o]
    w2t = np.ascontiguousarray(Wconv[:, C:].T)   # [c_in, o]
    w1tp = _pack4(w1t, C).astype(bf)
    w2tqp = _pack4(w2t, C).astype(bf)

    cfTs, comps, cf2ds, cfts = [], [], [], []
    for b in range(B):
        ct = np.asarray(content_feat[b], f).reshape(S, C)
        cfts.append(ct)
        cfTs.append(np.ascontiguousarray(ct.T))
        cf2ds.append(np.asarray(content_feat[b], f).reshape(C, S))
        comps.append([np.ascontiguousarray(
            np.asarray(components[n, b], f).reshape(S, C).T)
            for n in range(NC)])

    in_maps = []
    for core in range(N_CORES):
        b, hg = core // 4, core % 4
        sl = slice(128 * hg, 128 * (hg + 1))
        vsl = slice(C + 128 * hg, C + 128 * (hg + 1))
        qsl = slice(256 * hg, 256 * (hg + 1))
        posv = pos2 @ Wkv[:, vsl]            # [S, 128]
        # cf_tok strided blocks for the ctok tail: block 4u+kap is
        # cf_tok[u::2][128kap:128kap+128, sl]
        cfu = np.zeros((128, 8 * 128), f)
        for u in range(2):
            half = cfts[b][u::2]             # [512, C]
            for kap in range(4):
                cfu[:, 128 * (4 * u + kap):128 * (4 * u + kap + 1)] = \
                    half[128 * kap:128 * (kap + 1), sl]
        in_maps.append({
            **{f"cfp{h}": _pack4(np.ascontiguousarray(
                cfTs[b][:, 512 * h:512 * (h + 1)]), 512).astype(bf)
               for h in range(2)},
            **{f"comp0{h}": _pack4(np.ascontiguousarray(
                comps[b][0][:, 512 * h:512 * (h + 1)]), 512).astype(bf)
               for h in range(2)},
            **{f"compp{n}": _pack4(comps[b][n], S).astype(bf)
               for n in range(1, NC)},
            "cfup": cfu.astype(bf),
            "cf2dqp": _pack4(np.ascontiguousarray(cf2ds[b][:, qsl]),
                             256).astype(bf),
            "wqp": _pack4(np.ascontiguousarray(Wq[:, sl]), 128).astype(bf),
            "wkp": _pack4(np.ascontiguousarray(Wkv[:, sl]), 128).astype(bf),
            "wvp": _pack4(np.ascontiguousarray(Wkv[:, vsl]), 128).astype(bf),
            "posqp": np.ascontiguousarray((pos2 @ Wq[:, sl]).T).astype(bf),
            "poskp": np.ascontiguousarray((pos2 @ Wkv[:, sl]).T).astype(bf),
            "posvp": np.ascontiguousarray(
                posv.reshape(8, 128, 128).transpose(1, 0, 2)
                .reshape(128, S)).astype(bf),
            "w1tp": w1tp,
            "w2tqp": w2tqp,
            "wpslicep": np.ascontiguousarray(Wproj[sl, :]).astype(bf),
        })
    return in_maps


def _gather(res, inputs):
    f = np.float32
    Wconv = np.asarray(inputs["Wconv"], f)
    Wproj = np.asarray(inputs["Wproj"], f)
    bproj = np.asarray(inputs["bproj"], f)
    bconv = np.asarray(inputs["bconv"], f)
    pos2 = np.asarray(inputs["pos_emb"], f).reshape(S, C)
    W1 = Wconv[:, :C]
    w1sum = W1.sum(axis=1)
    const = np.zeros((C, S), f)
    for u in range(2):
        const[:, 512 * u:512 * (u + 1)] = \
            (W1 @ pos2[u::2]) @ Wproj + np.outer(w1sum, bproj)
    const += bconv[:, None]
    out = []
    for b in range(B):
        acc = const.copy()
        for hg in range(4):
            r = res.results[4 * b + hg]
            acc = acc + np.asarray(r["out_p"], f)
            q = np.asarray(r["qout"], f).reshape(128, 4, 256)
            acc[:, 256 * hg:256 * (hg + 1)] += (
                q.transpose(1, 0, 2).reshape(512, 256))
        out.append(acc)
    return np.stack(out, axis=0).reshape(B, C, H, W).astype(np.float32)


def _run(trace=False, **inputs):
    from concourse.bass_utils import run_bass_kernel_spmd

    nc = _build()
    in_maps = _shard_inputs(**inputs)
    res = run_bass_kernel_spmd(nc, in_maps, list(range(N_CORES)), trace=trace)
    return _gather(res, inputs), res


def kernel(**inputs):
    out, _ = _run(trace=False, **inputs)
    return out
